# revision 4
# baseline (speedup 1.0000x reference)
"""3-layer GCN (GCNConv x3) on 8 Trainium2 NeuronCores.

Strategy (graph/data parallel, per sharding hint):
  - Relabel nodes into 50176 slots = 392 blocks x 128; dst-blocks sharded
    8 ways (49 blocks/core). Per-block edge counts balanced by serpentine
    assignment on node in-degree.
  - Algebraic reorder: A(xW) = (Ax)W, so each layer is SpMM-then-GEMM with
    SpMM channel widths 116/256/2 (minimal gather volume).
  - SpMM on PE: sort edges by dst block, dma_gather source rows (bf16),
    segment-sum via matmul with per-tile one-hot scatter matrices S
    (S[e, j] = coeff[e] * (j == local_dst[e]), built on DVE with one fused
    tensor_scalar), accumulating in PSUM per 128-dst block.
  - deg/dinv and the GCN normalization coeffs are folded into S on host.
  - Layer boundaries (AllGather of the per-layer feature tables) are done
    host-side between the three launches (collectives unavailable here).
"""

import math
import numpy as np
import ml_dtypes

import concourse.bass as bass
import concourse.mybir as mybir
import concourse.tile as tile
from concourse import bacc
from concourse.bass_utils import run_bass_kernel_spmd
from concourse.library_config import mlp
from concourse.masks import make_identity

P = 128
N_NODES = 50000
N_EDGES = 1600000
IN_CH, HID, OUT = 116, 256, 2
N_CORES = 8
NSLOT = 50176            # 392 blocks * 128
NBLK = NSLOT // P        # 392
BLK_PER_CORE = NBLK // N_CORES   # 49
NPC = NSLOT // N_CORES   # 6272 slots per core
TAB_HALF = 32768         # int16 index limit per gather table view
BF16 = mybir.dt.bfloat16
F32 = mybir.dt.float32
I16 = mybir.dt.int16

_nc_cache = {}
TIMES = {}


# ---------------------------------------------------------------- host prep

def _wrap_idxs(idx):
    """[n] int -> [128, n/16] int16 (idx i at partition i%16, col i//16,
    replicated down the 8 Q7 core groups)."""
    n = len(idx)
    w = idx.reshape(n // 16, 16).T
    return np.ascontiguousarray(np.tile(w, (8, 1)).astype(np.int16))


def _prep(x, edge_index):
    src = np.asarray(edge_index[0], np.int64)
    dst = np.asarray(edge_index[1], np.int64)

    deg = (np.bincount(dst, minlength=N_NODES) + 1.0).astype(np.float32)
    dinv = (1.0 / np.sqrt(deg)).astype(np.float32)

    # serpentine assignment of nodes to blocks, by in-degree (balances
    # per-block edge counts incl. self loops)
    order = np.argsort(-deg, kind="stable")
    pos = np.arange(N_NODES)
    rounds, lane = pos // NBLK, pos % NBLK
    blk_of_sorted = np.where(rounds % 2 == 0, lane, NBLK - 1 - lane)
    blk = np.empty(N_NODES, np.int64)
    blk[order] = blk_of_sorted
    # rank within block -> slot
    bord = np.argsort(blk, kind="stable")
    sb = blk[bord]
    starts = np.searchsorted(sb, np.arange(NBLK))
    rank = np.empty(N_NODES, np.int64)
    rank[bord] = np.arange(N_NODES) - starts[sb]
    slot_of = (blk * P + rank).astype(np.int64)
    assert rank.max() < P

    # augmented edge list (real + self loops), in slot space
    es = slot_of[src]
    ed = slot_of[dst]
    coeff = dinv[src] * dinv[dst]
    self_s = slot_of
    self_c = dinv * dinv
    es = np.concatenate([es, self_s]).astype(np.int64)
    ed = np.concatenate([ed, self_s]).astype(np.int64)
    coeff = np.concatenate([coeff, self_c]).astype(np.float32)

    eblk = ed >> 7
    half = (es >= TAB_HALF).astype(np.int64)
    key = eblk * 2 + half
    sort_idx = np.argsort(key, kind="stable")
    es, ed, coeff, eblk, half = (a[sort_idx] for a in (es, ed, coeff, eblk, half))
    key = key[sort_idx]

    cnt = np.bincount(key, minlength=NBLK * 2).reshape(NBLK, 2)
    T0 = int(math.ceil(cnt[:, 0].max() / P))
    T1 = int(math.ceil(cnt[:, 1].max() / P))
    bounds = np.concatenate([[0], np.cumsum(cnt.reshape(-1))])

    n0, n1 = BLK_PER_CORE * T0 * P, BLK_PER_CORE * T1 * P
    ntile = BLK_PER_CORE * (T0 + T1)
    per_core = []
    for c in range(N_CORES):
        idx0 = np.zeros(n0, np.int64)
        idx1 = np.zeros(n1, np.int64)
        dstl = np.zeros(ntile * P, np.float32)
        cf = np.zeros(ntile * P, np.float32)
        for lb in range(BLK_PER_CORE):
            b = c * BLK_PER_CORE + lb
            s0, e0 = bounds[2 * b], bounds[2 * b + 1]
            s1, e1 = bounds[2 * b + 1], bounds[2 * b + 2]
            m0, m1 = e0 - s0, e1 - s1
            o0 = lb * T0 * P
            idx0[o0 : o0 + m0] = es[s0:e0]
            dstl[o0 : o0 + m0] = (ed[s0:e0] & 127).astype(np.float32)
            cf[o0 : o0 + m0] = coeff[s0:e0]
            o1 = lb * T1 * P
            idx1[o1 : o1 + m1] = es[s1:e1] - TAB_HALF
            g1 = n0 + o1
            dstl[g1 : g1 + m1] = (ed[s1:e1] & 127).astype(np.float32)
            cf[g1 : g1 + m1] = coeff[s1:e1]
        per_core.append({
            "ix0": _wrap_idxs(idx0),
            "ix1": _wrap_idxs(idx1),
            "dstl": np.ascontiguousarray(dstl.reshape(ntile, P).T),
            "cf": np.ascontiguousarray(cf.reshape(ntile, P).T),
        })

    xtab = np.zeros((NSLOT, P), np.float32)
    xtab[slot_of, :IN_CH] = np.asarray(x, np.float32)
    xtab = xtab.astype(ml_dtypes.bfloat16)
    return per_core, slot_of, xtab, T0, T1


# ------------------------------------------------------------- common pieces

IOTA = np.ascontiguousarray(
    np.broadcast_to(np.arange(P, dtype=np.float32), (P, P))
).astype(ml_dtypes.bfloat16)


def _load_consts(nc, tc, cpool, T0, T1, with_ident=False):
    ntile = BLK_PER_CORE * (T0 + T1)
    n0c, n1c = BLK_PER_CORE * T0 * 8, BLK_PER_CORE * T1 * 8
    iota_d = nc.dram_tensor("iota", [P, P], BF16, kind="ExternalInput")
    ix0_d = nc.dram_tensor("ix0", [P, n0c], I16, kind="ExternalInput")
    ix1_d = nc.dram_tensor("ix1", [P, n1c], I16, kind="ExternalInput")
    dstl_d = nc.dram_tensor("dstl", [P, ntile], F32, kind="ExternalInput")
    cf_d = nc.dram_tensor("cf", [P, ntile], F32, kind="ExternalInput")
    iota_t = cpool.tile([P, P], BF16)
    nc.sync.dma_start(out=iota_t[:], in_=iota_d.ap())
    ix0_t = cpool.tile([P, n0c], I16)
    nc.sync.dma_start(out=ix0_t[:], in_=ix0_d.ap())
    ix1_t = cpool.tile([P, n1c], I16)
    nc.sync.dma_start(out=ix1_t[:], in_=ix1_d.ap())
    dstl_t = cpool.tile([P, ntile], F32)
    nc.sync.dma_start(out=dstl_t[:], in_=dstl_d.ap())
    cf_t = cpool.tile([P, ntile], F32)
    nc.sync.dma_start(out=cf_t[:], in_=cf_d.ap())
    ident_t = None
    if with_ident:
        ident_t = cpool.tile([P, P], BF16)
        make_identity(nc, ident_t[:])
    return iota_t, ix0_t, ix1_t, dstl_t, cf_t, ident_t


def _build_S(nc, pool, iota_t, dstl_t, cf_t, g):
    S = pool.tile([P, P], BF16, tag="S")
    nc.vector.tensor_scalar(
        out=S[:], in0=iota_t[:],
        scalar1=dstl_t[:, g : g + 1], scalar2=cf_t[:, g : g + 1],
        op0=mybir.AluOpType.is_equal, op1=mybir.AluOpType.mult,
    )
    return S


def _chunks(chunk):
    out = []
    b = 0
    while b < BLK_PER_CORE:
        nb = min(chunk, BLK_PER_CORE - b)
        out.append((b, nb))
        b += nb
    return out


# --------------------------------------------------------------- L1 builder

def _build_l1(T0, T1):
    """SpMM(x, 128ch, Way A -> agg1T [128, NPC]) + GEMM -> h1T [256, NPC]."""
    nc = bacc.Bacc("TRN2", target_bir_lowering=False, debug=False,
                   num_devices=N_CORES)
    tab_d = nc.dram_tensor("xtab", [NSLOT, P], BF16, kind="ExternalInput")
    w1_d = nc.dram_tensor("w1", [P, HID], BF16, kind="ExternalInput")
    b1_d = nc.dram_tensor("b1", [P, 2], F32, kind="ExternalInput")
    out_d = nc.dram_tensor("h1T", [HID, NPC], BF16, kind="ExternalOutput")
    CHUNK = 4

    with tile.TileContext(nc) as tc:
        with (
            tc.tile_pool(name="const", bufs=1) as cpool,
            tc.tile_pool(name="big", bufs=1) as bigpool,
            tc.tile_pool(name="m0", bufs=2) as m0pool,
            tc.tile_pool(name="m1", bufs=2) as m1pool,
            tc.tile_pool(name="S", bufs=6) as spool,
            tc.tile_pool(name="ps", bufs=2, space="PSUM") as pspool,
            tc.tile_pool(name="psg", bufs=2, space="PSUM") as psgpool,
        ):
            nc.gpsimd.load_library(mlp)
            iota_t, ix0_t, ix1_t, dstl_t, cf_t, _ = _load_consts(
                nc, tc, cpool, T0, T1)
            w1_t = cpool.tile([P, HID], BF16)
            nc.sync.dma_start(out=w1_t[:], in_=w1_d.ap())
            b1_t = cpool.tile([P, 2], F32)
            nc.sync.dma_start(out=b1_t[:], in_=b1_d.ap())

            agg1T = bigpool.tile([P, NPC], BF16)

            for b0, nb in _chunks(CHUNK):
                msg0 = m0pool.tile([P, CHUNK * T0, P], BF16, tag="msg0")
                nc.gpsimd.dma_gather(
                    msg0[:, : nb * T0, :], tab_d.ap()[0:TAB_HALF, :],
                    ix0_t[:, b0 * T0 * 8 : (b0 + nb) * T0 * 8],
                    nb * T0 * P, nb * T0 * P, P, single_packet=False)
                msg1 = m1pool.tile([P, CHUNK * T1, P], BF16, tag="msg1")
                nc.gpsimd.dma_gather(
                    msg1[:, : nb * T1, :], tab_d.ap()[TAB_HALF:NSLOT, :],
                    ix1_t[:, b0 * T1 * 8 : (b0 + nb) * T1 * 8],
                    nb * T1 * P, nb * T1 * P, P, single_packet=False)
                for lb in range(nb):
                    b = b0 + lb
                    ps = pspool.tile([P, P], F32, tag="ps")
                    for t in range(T0 + T1):
                        if t < T0:
                            m = msg0[:, lb * T0 + t, :]
                            g = b * T0 + t
                        else:
                            m = msg1[:, lb * T1 + (t - T0), :]
                            g = BLK_PER_CORE * T0 + b * T1 + (t - T0)
                        S = _build_S(nc, spool, iota_t, dstl_t, cf_t, g)
                        # Way A: psum[ch, dst] += msg.T @ S
                        nc.tensor.matmul(ps[:], lhsT=m, rhs=S[:],
                                         start=(t == 0), stop=(t == T0 + T1 - 1))
                    nc.vector.tensor_copy(
                        out=agg1T[:, b * P : (b + 1) * P], in_=ps[:])

            # GEMM: h1T[m] = relu(W1[:, m].T @ agg1T + b1[m])
            NT_N = math.ceil(NPC / 512)
            for m in range(2):
                h1T = bigpool.tile([P, NPC], BF16, tag=f"h1T{m}")
                for n in range(NT_N):
                    ns = n * 512
                    nsz = min(512, NPC - ns)
                    psg = psgpool.tile([P, 512], F32, tag="psg")
                    nc.tensor.matmul(
                        psg[:, :nsz], lhsT=w1_t[:, m * P : (m + 1) * P],
                        rhs=agg1T[:, ns : ns + nsz], start=True, stop=True)
                    nc.scalar.activation(
                        out=h1T[:, ns : ns + nsz], in_=psg[:, :nsz],
                        func=mybir.ActivationFunctionType.Relu,
                        bias=b1_t[:, m : m + 1], scale=1.0)
                nc.sync.dma_start(out=out_d.ap()[m * P : (m + 1) * P, :],
                                  in_=h1T[:])
    nc.compile()
    return nc


# --------------------------------------------------------------- L2 builder

def _build_l2(T0, T1):
    """SpMM(t2, 256ch, Way B) -> transpose -> GEMM W2+relu -> GEMM W3
    -> t3T [2, NPC] f32."""
    nc = bacc.Bacc("TRN2", target_bir_lowering=False, debug=False,
                   num_devices=N_CORES)
    tab_d = nc.dram_tensor("t2", [NSLOT, HID], BF16, kind="ExternalInput")
    w2_d = nc.dram_tensor("w2", [HID, HID], BF16, kind="ExternalInput")
    b2_d = nc.dram_tensor("b2", [P, 2], F32, kind="ExternalInput")
    w3_d = nc.dram_tensor("w3", [HID, OUT], BF16, kind="ExternalInput")
    out_d = nc.dram_tensor("t3T", [OUT, NPC], F32, kind="ExternalOutput")
    CHUNK = 1

    with tile.TileContext(nc) as tc:
        with (
            tc.tile_pool(name="const", bufs=1) as cpool,
            tc.tile_pool(name="big", bufs=1) as bigpool,
            tc.tile_pool(name="m0", bufs=2) as m0pool,
            tc.tile_pool(name="m1", bufs=2) as m1pool,
            tc.tile_pool(name="S", bufs=6) as spool,
            tc.tile_pool(name="work", bufs=3) as wpool,
            tc.tile_pool(name="ps", bufs=2, space="PSUM") as pspool,
            tc.tile_pool(name="pst", bufs=2, space="PSUM") as pstpool,
            tc.tile_pool(name="psg", bufs=2, space="PSUM") as psgpool,
            tc.tile_pool(name="psg3", bufs=2, space="PSUM") as psg3pool,
        ):
            nc.gpsimd.load_library(mlp)
            iota_t, ix0_t, ix1_t, dstl_t, cf_t, ident_t = _load_consts(
                nc, tc, cpool, T0, T1, with_ident=True)
            w2_t = []
            for k in range(HID // P):
                wk = cpool.tile([P, HID], BF16, tag=f"w2k{k}")
                nc.sync.dma_start(out=wk[:], in_=w2_d.ap()[k * P:(k + 1) * P, :])
                w2_t.append(wk)
            b2_t = cpool.tile([P, 2], F32)
            nc.sync.dma_start(out=b2_t[:], in_=b2_d.ap())
            w3_t = []
            for k in range(HID // P):
                wk = cpool.tile([P, OUT], BF16, tag=f"w3k{k}")
                nc.sync.dma_start(out=wk[:], in_=w3_d.ap()[k * P:(k + 1) * P, :])
                w3_t.append(wk)

            agg2T_a = bigpool.tile([P, NPC], BF16)
            agg2T_b = bigpool.tile([P, NPC], BF16)

            for b0, nb in _chunks(CHUNK):
                msg0 = m0pool.tile([P, CHUNK * T0, HID], BF16, tag="msg0")
                nc.gpsimd.dma_gather(
                    msg0[:, : nb * T0, :], tab_d.ap()[0:TAB_HALF, :],
                    ix0_t[:, b0 * T0 * 8 : (b0 + nb) * T0 * 8],
                    nb * T0 * P, nb * T0 * P, HID, single_packet=False)
                msg1 = m1pool.tile([P, CHUNK * T1, HID], BF16, tag="msg1")
                nc.gpsimd.dma_gather(
                    msg1[:, : nb * T1, :], tab_d.ap()[TAB_HALF:NSLOT, :],
                    ix1_t[:, b0 * T1 * 8 : (b0 + nb) * T1 * 8],
                    nb * T1 * P, nb * T1 * P, HID, single_packet=False)
                for lb in range(nb):
                    b = b0 + lb
                    ps = pspool.tile([P, HID], F32, tag="ps")
                    for t in range(T0 + T1):
                        if t < T0:
                            m = msg0[:, lb * T0 + t, :]
                            g = b * T0 + t
                        else:
                            m = msg1[:, lb * T1 + (t - T0), :]
                            g = BLK_PER_CORE * T0 + b * T1 + (t - T0)
                        S = _build_S(nc, spool, iota_t, dstl_t, cf_t, g)
                        nc.tensor.matmul(ps[:], lhsT=S[:], rhs=m,
                                         start=(t == 0), stop=(t == T0 + T1 - 1))
                    agg2 = wpool.tile([P, HID], BF16, tag="agg2")
                    nc.vector.tensor_copy(out=agg2[:], in_=ps[:])
                    for h, aggT in ((0, agg2T_a), (1, agg2T_b)):
                        pst = pstpool.tile([P, P], BF16, tag="pst")
                        nc.tensor.transpose(
                            pst[:], agg2[:, h * P : (h + 1) * P], ident_t[:])
                        nc.vector.tensor_copy(
                            out=aggT[:, b * P : (b + 1) * P], in_=pst[:])

            # GEMM h2T = relu(W2.T @ agg2T + b2); t3T = W3.T @ h2T
            NT_N = math.ceil(NPC / 512)
            h2T = [bigpool.tile([P, NPC], BF16, tag=f"h2T{m}", name=f"h2T{m}") for m in range(2)]
            for m in range(2):
                for n in range(NT_N):
                    ns = n * 512
                    nsz = min(512, NPC - ns)
                    psg = psgpool.tile([P, 512], F32, tag="psg")
                    for k, aggT in ((0, agg2T_a), (1, agg2T_b)):
                        nc.tensor.matmul(
                            psg[:, :nsz],
                            lhsT=w2_t[k][:, m * P : (m + 1) * P],
                            rhs=aggT[:, ns : ns + nsz],
                            start=(k == 0), stop=(k == 1))
                    nc.scalar.activation(
                        out=h2T[m][:, ns : ns + nsz], in_=psg[:, :nsz],
                        func=mybir.ActivationFunctionType.Relu,
                        bias=b2_t[:, m : m + 1], scale=1.0)
            t3T = bigpool.tile([OUT, NPC], F32)
            for n in range(NT_N):
                ns = n * 512
                nsz = min(512, NPC - ns)
                psg3 = psg3pool.tile([OUT, 512], F32, tag="psg3")
                for k in range(2):
                    nc.tensor.matmul(
                        psg3[:, :nsz], lhsT=w3_t[k][:],
                        rhs=h2T[k][:, ns : ns + nsz],
                        start=(k == 0), stop=(k == 1))
                nc.vector.tensor_copy(out=t3T[:, ns : ns + nsz],
                                      in_=psg3[:, :nsz])
            nc.sync.dma_start(out=out_d.ap(), in_=t3T[:])
    nc.compile()
    return nc


# --------------------------------------------------------------- L3 builder

def _build_l3(T0, T1):
    """SpMM(t3pad, 2ch, Way B) + b3 -> outb [128, 49*2] f32."""
    nc = bacc.Bacc("TRN2", target_bir_lowering=False, debug=False,
                   num_devices=N_CORES)
    tab_d = nc.dram_tensor("t3pad", [NSLOT, P], BF16, kind="ExternalInput")
    b3_d = nc.dram_tensor("b3rep", [P, OUT], F32, kind="ExternalInput")
    out_d = nc.dram_tensor("outb", [P, BLK_PER_CORE * OUT], F32,
                           kind="ExternalOutput")
    CHUNK = 4

    with tile.TileContext(nc) as tc:
        with (
            tc.tile_pool(name="const", bufs=1) as cpool,
            tc.tile_pool(name="big", bufs=1) as bigpool,
            tc.tile_pool(name="m0", bufs=2) as m0pool,
            tc.tile_pool(name="m1", bufs=2) as m1pool,
            tc.tile_pool(name="S", bufs=6) as spool,
            tc.tile_pool(name="ps", bufs=2, space="PSUM") as pspool,
        ):
            nc.gpsimd.load_library(mlp)
            iota_t, ix0_t, ix1_t, dstl_t, cf_t, _ = _load_consts(
                nc, tc, cpool, T0, T1)
            b3_t = cpool.tile([P, OUT], F32)
            nc.sync.dma_start(out=b3_t[:], in_=b3_d.ap())
            ressb = bigpool.tile([P, BLK_PER_CORE * OUT], F32)

            for b0, nb in _chunks(CHUNK):
                msg0 = m0pool.tile([P, CHUNK * T0, P], BF16, tag="msg0")
                nc.gpsimd.dma_gather(
                    msg0[:, : nb * T0, :], tab_d.ap()[0:TAB_HALF, :],
                    ix0_t[:, b0 * T0 * 8 : (b0 + nb) * T0 * 8],
                    nb * T0 * P, nb * T0 * P, P, single_packet=False)
                msg1 = m1pool.tile([P, CHUNK * T1, P], BF16, tag="msg1")
                nc.gpsimd.dma_gather(
                    msg1[:, : nb * T1, :], tab_d.ap()[TAB_HALF:NSLOT, :],
                    ix1_t[:, b0 * T1 * 8 : (b0 + nb) * T1 * 8],
                    nb * T1 * P, nb * T1 * P, P, single_packet=False)
                for lb in range(nb):
                    b = b0 + lb
                    ps = pspool.tile([P, OUT], F32, tag="ps")
                    for t in range(T0 + T1):
                        if t < T0:
                            m = msg0[:, lb * T0 + t, 0:OUT]
                            g = b * T0 + t
                        else:
                            m = msg1[:, lb * T1 + (t - T0), 0:OUT]
                            g = BLK_PER_CORE * T0 + b * T1 + (t - T0)
                        S = _build_S(nc, spool, iota_t, dstl_t, cf_t, g)
                        nc.tensor.matmul(ps[:], lhsT=S[:], rhs=m,
                                         start=(t == 0), stop=(t == T0 + T1 - 1))
                    nc.vector.tensor_tensor(
                        out=ressb[:, b * OUT : (b + 1) * OUT],
                        in0=ps[:], in1=b3_t[:], op=mybir.AluOpType.add)
            nc.sync.dma_start(out=out_d.ap(), in_=ressb[:])
    nc.compile()
    return nc


# ------------------------------------------------------------------- driver

def kernel(x, edge_index, W1, b1, W2, b2, W3, b3):
    x = np.asarray(x, np.float32)
    per_core, slot_of, xtab, T0, T1 = _prep(x, edge_index)

    key = (T0, T1)
    if key not in _nc_cache:
        _nc_cache[key] = (_build_l1(T0, T1), _build_l2(T0, T1),
                          _build_l3(T0, T1))
    nc1, nc2, nc3 = _nc_cache[key]

    w1p = np.zeros((P, HID), np.float32)
    w1p[:IN_CH] = np.asarray(W1, np.float32)
    w1p = w1p.astype(ml_dtypes.bfloat16)
    b1c = np.ascontiguousarray(np.asarray(b1, np.float32).reshape(2, P).T)
    w2b = np.asarray(W2, np.float32).astype(ml_dtypes.bfloat16)
    b2c = np.ascontiguousarray(np.asarray(b2, np.float32).reshape(2, P).T)
    w3b = np.asarray(W3, np.float32).astype(ml_dtypes.bfloat16)
    b3rep = np.ascontiguousarray(
        np.broadcast_to(np.asarray(b3, np.float32), (P, OUT)))

    core_ids = list(range(N_CORES))

    ins1 = [{
        "xtab": xtab, "w1": w1p, "b1": b1c, "iota": IOTA,
        **{k: pc[k] for k in ("ix0", "ix1", "dstl", "cf")},
    } for pc in per_core]
    import time as _time
    _t = _time.perf_counter()
    r1 = run_bass_kernel_spmd(nc1, ins1, core_ids=core_ids)
    TIMES["l1"] = _time.perf_counter() - _t
    t2 = np.concatenate([r["h1T"].T for r in r1.results], axis=0)
    t2 = np.ascontiguousarray(t2)  # [NSLOT, 256] bf16

    ins2 = [{
        "t2": t2, "w2": w2b, "b2": b2c, "w3": w3b, "iota": IOTA,
        **{k: pc[k] for k in ("ix0", "ix1", "dstl", "cf")},
    } for pc in per_core]
    _t = _time.perf_counter()
    r2 = run_bass_kernel_spmd(nc2, ins2, core_ids=core_ids)
    TIMES["l2"] = _time.perf_counter() - _t
    t3 = np.concatenate([r["t3T"].T for r in r2.results], axis=0)  # [NSLOT,2] f32
    t3pad = np.zeros((NSLOT, P), np.float32)
    t3pad[:, :OUT] = t3
    t3pad = t3pad.astype(ml_dtypes.bfloat16)

    ins3 = [{
        "t3pad": t3pad, "b3rep": b3rep, "iota": IOTA,
        **{k: pc[k] for k in ("ix0", "ix1", "dstl", "cf")},
    } for pc in per_core]
    _t = _time.perf_counter()
    r3 = run_bass_kernel_spmd(nc3, ins3, core_ids=core_ids)
    TIMES["l3"] = _time.perf_counter() - _t

    out_slots = np.concatenate(
        [r["outb"].reshape(P, BLK_PER_CORE, OUT).transpose(1, 0, 2)
         .reshape(NPC, OUT) for r in r3.results], axis=0)  # [NSLOT, 2]
    return np.ascontiguousarray(out_slots[slot_of]).astype(np.float32)
